# revision 1
# baseline (speedup 1.0000x reference)
"""Trainium2 Bass kernel for nn_E3ConvLayer (gnn_message_passing).

The reference reduces to (l>=1 spherical harmonics are dead code — only
W[:, :1] is used and Y[:, 0] == 1/sqrt(4*pi) is a constant; pos is unused):

  r0(e)  = softplus(nbr_fea[e] @ w1 + b1) @ w2[:, 0] + b2[0]
  w(e)   = r0(e) / (sqrt(4*pi) * sqrt(C) * M)
  pre[n] = sum_m w(n, m) * atom_fea[nbr_idx[n, m]]
  x      = atom_fea + pre @ tp_w
  out    = softplus(gamma * (x - mean(x)) * rsqrt(var(x) + 1e-5) + beta)

Design (8 cores, nodes sharded 6250/core, padded to 6272 = 49*128):
  * radial MLP on PE: 2x64 block-diagonal packing -> K=128 matmuls, N=512;
    softplus as Ln(Exp(x + b1) + 1) (both live in one ACT table set).
  * r0 via replicated-lhsT matmuls -> PSUM [128, 512] chunks (every
    partition holds the full row, so DVE can consume it directly).
  * neighbor features arrive TRANSPOSED via dma_gather(transpose=True)
    from per-slab index-remapped bf16 tables (int16 index limit).
  * aggregation on DVE: Gw^T = G^T * (r0 + b2c) in place, then a strided
    tensor_reduce sums each node's 12 columns -> pre^T.
  * agg = tp_w^T @ pre^T on PE; x = atom^T + agg with fused global-sum
    accumulators; BN stats AllReduce'd (2x128 floats); final
    softplus(gamma' x + beta') as one Exp + one Ln over the whole shard.
Host pre-transposes/pads inputs and post-transposes the output.
"""

import os
import sys
import numpy as np

sys.path.insert(0, "/opt/trn_rl_repo")

N_TOTAL, M, C, F = 50000, 12, 128, 64
N_CORES = 8
P = 128

_SQRT4PI = float(np.sqrt(4.0 * np.pi))
ALPHA = np.float32(1.0 / (_SQRT4PI * np.sqrt(C) * M))

_DBG = set(os.environ.get("KDBG", "").split(","))


def _plan(nloc):
    """Static shapes/chunking for one core."""
    npad = ((nloc + 127) // 128) * 128
    nt = npad // 128                  # 128-node tiles
    e_pad = npad * M                  # real edge slots (n-major)
    nchunk = e_pad // 512             # 512-edge mm2 chunks
    nblk = (nchunk + 1) // 2          # 1024-edge mm1 blocks
    ngrp = (nblk + 3) // 4            # 4096-edge Exp/Ln groups
    nt_a = 4 * (nt // 8) if nt >= 8 else max(1, nt // 2)
    chunks = []                       # (slab, tile_start, ntiles)
    for s, (t0, t1) in enumerate([(0, nt_a), (nt_a, nt)]):
        ts = t0
        while ts < t1:
            n = min(6, t1 - ts)
            chunks.append((s, ts, n))
            ts += n
    return dict(npad=npad, nt=nt, e_pad=e_pad, nchunk=nchunk, nblk=nblk,
                ngrp=ngrp, nt_a=nt_a, chunks=chunks)


def build_bass(nloc, u_sizes, b2c, reps=1, num_devices=N_CORES):
    import concourse.bacc as bacc
    import concourse.tile as tile
    from concourse import mybir
    from contextlib import ExitStack

    f32 = mybir.dt.float32
    bf16 = mybir.dt.bfloat16
    i16 = mybir.dt.int16
    AF = mybir.ActivationFunctionType
    ALU = mybir.AluOpType

    pl = _plan(nloc)
    npad, e_pad, ngrp = pl["npad"], pl["e_pad"], pl["ngrp"]
    chunks = pl["chunks"]

    nc = bacc.Bacc("TRN2", target_bir_lowering=False, debug=False,
                   enable_asserts=True, num_devices=num_devices)

    # ---- DRAM parameters -------------------------------------------------
    nbrT = nc.dram_tensor("nbrT", [P, ngrp * 2048], bf16, kind="ExternalInput").ap()
    tabs = [nc.dram_tensor(f"tab{s}", [u_sizes[s], C], bf16, kind="ExternalInput").ap()
            for s in range(len(u_sizes))]
    idx16 = nc.dram_tensor("idx16", [P, e_pad // 16], i16, kind="ExternalInput").ap()
    atomT = nc.dram_tensor("atomT", [P, npad], f32, kind="ExternalInput").ap()
    cbf = nc.dram_tensor("cbf", [P, 256], bf16, kind="ExternalInput").ap()
    cf32 = nc.dram_tensor("cf32", [P, 131], f32, kind="ExternalInput").ap()
    outT = nc.dram_tensor("outT", [P, npad], f32, kind="ExternalOutput").ap()
    cc_in = nc.dram_tensor("cc_in", [P, 2], f32).ap()
    cc_out = nc.dram_tensor("cc_out", [P, 2], f32, addr_space="Shared").ap()

    with tile.TileContext(nc) as tc, ExitStack() as ctx:
        const_pool = ctx.enter_context(tc.tile_pool(name="const", bufs=1))
        nbr_pool = ctx.enter_context(tc.tile_pool(name="nbr", bufs=2))
        s_pool = ctx.enter_context(tc.tile_pool(name="spak", bufs=6))
        e_pool = ctx.enter_context(tc.tile_pool(name="epak", bufs=2))
        g_pool = ctx.enter_context(tc.tile_pool(name="gath", bufs=2))
        pret_pool = ctx.enter_context(tc.tile_pool(name="pret", bufs=1))
        big_pool = ctx.enter_context(tc.tile_pool(name="big", bufs=1))
        stat_pool = ctx.enter_context(tc.tile_pool(name="stat", bufs=1))

        psum_h = ctx.enter_context(tc.tile_pool(name="ph", bufs=1, space="PSUM"))
        psum_r0 = ctx.enter_context(tc.tile_pool(name="pr0", bufs=1, space="PSUM"))
        psum_agg = ctx.enter_context(tc.tile_pool(name="pagg", bufs=1, space="PSUM"))

        # ---- constant preloads ------------------------------------------
        cb_sb = const_pool.tile([P, 256], bf16)
        nc.sync.dma_start(cb_sb[:], cbf[:])
        w1_sb = cb_sb[:, 0:128]          # blockdiag(w1, w1)
        w2_sb = cb_sb[:, 128:256]        # [w2c*alpha] replicated to 128 cols, x2
        cf_sb = const_pool.tile([P, 131], f32)
        nc.sync.dma_start(cf_sb[:], cf32[:])
        b1_sb = cf_sb[:, 0:1]
        gam_sb = cf_sb[:, 1:2]
        bet_sb = cf_sb[:, 2:3]
        tpw_sb = cf_sb[:, 3:131]         # tp_w (f32)
        idx_sb = const_pool.tile([P, e_pad // 16], i16)
        nc.sync.dma_start(idx_sb[:], idx16[:])

        x_sb = big_pool.tile([P, npad], f32, tag="x")
        pret_pool_t = pret_pool.tile([P, npad], f32, tag="pret")

        for rep in range(reps):
            stats = stat_pool.tile([P, 16], f32, tag="st")
            pret_sb = pret_pool_t

            # radial MLP groups are emitted lazily (ring of s-tiles) just
            # before the node-tiles that consume them
            s_tiles = {}
            next_g = [0]

            def emit_radial_through(g_need):
                while next_g[0] <= g_need:
                    g = next_g[0]
                    nbr_sb = nbr_pool.tile([P, 2048], bf16, tag="nbr")
                    nc.sync.dma_start(nbr_sb[:], nbrT[:, g * 2048:(g + 1) * 2048])
                    h_ps = psum_h.tile([P, 2048], f32, tag="h")
                    for b in range(4):
                        nc.tensor.matmul(h_ps[:, b * 512:(b + 1) * 512], w1_sb,
                                         nbr_sb[:, b * 512:(b + 1) * 512],
                                         start=True, stop=True)
                    e_sb = e_pool.tile([P, 2048], f32, tag="e")
                    nc.scalar.activation(e_sb[:], h_ps[:], AF.Exp, bias=b1_sb)
                    s_sb = s_pool.tile([P, 2048], bf16, tag="s")
                    nc.scalar.activation(s_sb[:], e_sb[:], AF.Ln, bias=1.0)
                    s_tiles[g] = s_sb
                    next_g[0] += 1

            # ======= gather + r0 + weight + reduce -> pre^T ==============
            for (slab, ts, gnt) in chunks:
                ne = gnt * 1536
                e0 = ts * 1536
                gath_sb = g_pool.tile([P, 9216], bf16, tag="g")
                gt = gath_sb[:, :ne]
                if "nogather" in _DBG:
                    nc.vector.memset(gt, 1.0)
                else:
                    gv = gt.rearrange("p (k d) -> p k d", d=ne)
                    nc.gpsimd.dma_gather(
                        out_ap=gv, in_ap=tabs[slab][:],
                        idxs_ap=idx_sb[:, e0 // 16:(e0 + ne) // 16],
                        num_idxs=ne, num_idxs_reg=ne,
                        elem_size=C, transpose=True, single_packet=False)
                emit_radial_through(min((3 * (ts + gnt) - 1) // 8, ngrp - 1))
                for t in range(ts, ts + gnt):
                    r0_ps = psum_r0.tile([P, 1536], f32, tag="r0")
                    for k in range(3):
                        ch = 3 * t + k               # global 512-edge chunk
                        blk = ch // 2
                        half = ch % 2
                        s_sb = s_tiles[blk // 4]
                        scol = (blk % 4) * 512
                        nc.tensor.matmul(
                            r0_ps[:, k * 512:(k + 1) * 512],
                            w2_sb[64 * half:64 * (half + 1), :],
                            s_sb[64 * half:64 * (half + 1), scol:scol + 512],
                            start=True, stop=True)
                    off = (t - ts) * 1536
                    nc.vector.scalar_tensor_tensor(
                        out=gt[:, off:off + 1536], in0=r0_ps[:, :],
                        scalar=float(b2c), in1=gt[:, off:off + 1536],
                        op0=ALU.add, op1=ALU.mult)
                rv = gt.rearrange("p (n m) -> p n m", m=M)
                nc.vector.reduce_sum(pret_sb[:, ts * P:(ts + gnt) * P], rv,
                                     axis=mybir.AxisListType.X)

            # ========= agg = tp_w^T @ pre^T ; x ; global stats ===========
            agg_big = big_pool.tile([P, npad], f32, tag="aggb")
            nco = (npad + 511) // 512
            for i in range(nco):
                n0, n1 = i * 512, min((i + 1) * 512, npad)
                agg_ps = psum_agg.tile([P, 512], f32, tag="agg")
                nc.tensor.matmul(agg_ps[:, :n1 - n0], tpw_sb,
                                 pret_sb[:, n0:n1], start=True, stop=True)
                nc.vector.tensor_copy(agg_big[:, n0:n1], agg_ps[:, :n1 - n0])
            atom_sb = big_pool.tile([P, npad], f32, tag="atom")
            nc.sync.dma_start(atom_sb[:], atomT[:])
            nc.vector.scalar_tensor_tensor(
                out=x_sb[:, :nloc], in0=agg_big[:, :nloc], scalar=1.0,
                in1=atom_sb[:, :nloc], op0=ALU.mult, op1=ALU.add,
                accum_out=stats[:, 0:1])
            if nloc < npad:
                nc.vector.scalar_tensor_tensor(
                    out=x_sb[:, nloc:], in0=agg_big[:, nloc:], scalar=1.0,
                    in1=atom_sb[:, nloc:], op0=ALU.mult, op1=ALU.add)
            nc.vector.scalar_tensor_tensor(
                out=agg_big[:, :nloc], in0=x_sb[:, :nloc], scalar=1.0,
                in1=x_sb[:, :nloc], op0=ALU.mult, op1=ALU.mult,
                accum_out=stats[:, 1:2])

            # ---- global BN stats over all cores ----
            cc_sb = stat_pool.tile([P, 2], f32, tag="cc")
            nc.vector.tensor_copy(cc_sb[:], stats[:, 0:2])
            nc.sync.dma_start(cc_in[:], cc_sb[:])
            if "nocc" in _DBG:
                nc.sync.dma_start(cc_out[:], cc_in[:])
            else:
                nc.gpsimd.collective_compute(
                    "AllReduce", ALU.add,
                    replica_groups=[list(range(num_devices))],
                    ins=[cc_in[:]], outs=[cc_out[:]])
            st_sb = stat_pool.tile([P, 2], f32, tag="stg")
            nc.sync.dma_start(st_sb[:], cc_out[:])

            # mean = sx/N ; var = sq/N - mean^2 ; rstd = exp(-0.5 ln(var+eps))
            w_sb = stat_pool.tile([P, 8], f32, tag="w")
            inv_n = 1.0 / float(nloc * num_devices)
            nc.vector.tensor_scalar_mul(w_sb[:, 0:1], st_sb[:, 0:1], inv_n)
            nc.vector.tensor_scalar_mul(w_sb[:, 1:2], st_sb[:, 1:2], inv_n)
            nc.vector.scalar_tensor_tensor(
                out=w_sb[:, 2:3], in0=w_sb[:, 0:1], scalar=-1.0,
                in1=w_sb[:, 0:1], op0=ALU.mult, op1=ALU.mult)  # -mean^2
            nc.vector.scalar_tensor_tensor(
                out=w_sb[:, 1:2], in0=w_sb[:, 1:2], scalar=1e-5,
                in1=w_sb[:, 2:3], op0=ALU.add, op1=ALU.add)    # var + eps
            nc.scalar.activation(w_sb[:, 2:3], w_sb[:, 1:2], AF.Ln)
            nc.scalar.activation(w_sb[:, 3:4], w_sb[:, 2:3], AF.Exp, scale=-0.5)
            # Newton: y <- y * (1.5 - 0.5*(var+eps)*y^2)
            nc.vector.tensor_tensor(w_sb[:, 4:5], w_sb[:, 3:4], w_sb[:, 3:4], op=ALU.mult)
            nc.vector.tensor_tensor(w_sb[:, 4:5], w_sb[:, 4:5], w_sb[:, 1:2], op=ALU.mult)
            nc.vector.tensor_scalar(w_sb[:, 4:5], w_sb[:, 4:5], -0.5, 1.5,
                                    op0=ALU.mult, op1=ALU.add)
            nc.vector.tensor_tensor(w_sb[:, 3:4], w_sb[:, 3:4], w_sb[:, 4:5], op=ALU.mult)
            # gamma' = gamma * rstd ; beta' = beta - mean * gamma'
            nc.vector.tensor_tensor(w_sb[:, 5:6], gam_sb, w_sb[:, 3:4], op=ALU.mult)
            nc.vector.tensor_tensor(w_sb[:, 6:7], w_sb[:, 0:1], w_sb[:, 5:6], op=ALU.mult)
            nc.vector.tensor_tensor(w_sb[:, 6:7], bet_sb, w_sb[:, 6:7], op=ALU.subtract)

            # ---- final: out = softplus(gamma' * x + beta') ----
            e2_sb = big_pool.tile([P, npad], f32, tag="atom")  # reuse atom slot
            nc.scalar.activation(e2_sb[:], x_sb[:], AF.Exp,
                                 bias=w_sb[:, 6:7], scale=w_sb[:, 5:6])
            o_sb = big_pool.tile([P, npad], f32, tag="aggb")   # reuse agg slot
            nc.scalar.activation(o_sb[:], e2_sb[:], AF.Ln, bias=1.0)
            nc.sync.dma_start(outT[:], o_sb[:])

    nc.compile()
    return nc


def prep_core_inputs(core, nloc, atom_fea, nbr_fea, nbr_idx, w1, b1, w2, b2,
                     tp_w, bn_gamma, bn_beta):
    """Host-side layout prep for one core."""
    import ml_dtypes
    bf = ml_dtypes.bfloat16

    pl = _plan(nloc)
    npad, e_pad, ngrp = pl["npad"], pl["e_pad"], pl["ngrp"]
    nt, nt_a = pl["nt"], pl["nt_a"]

    n0 = core * nloc
    nbr = np.zeros((npad, M, F), np.float32)
    nbr[:nloc] = nbr_fea[n0:n0 + nloc]
    src = np.zeros((npad, M), np.int64)
    src[:nloc] = nbr_idx[n0:n0 + nloc]

    # radial stream (n-major edges), packed 2x64 rows per 1024-edge block
    X = nbr.reshape(e_pad, F)
    Xp = np.zeros((ngrp * 4096, F), np.float32)
    Xp[:e_pad] = X
    blocks = Xp.reshape(ngrp * 4, 2, 512, F)
    nbrT = np.empty((P, ngrp * 2048), bf)
    nbrT[:F] = blocks[:, 0].transpose(2, 0, 1).reshape(F, -1).astype(bf)
    nbrT[F:] = blocks[:, 1].transpose(2, 0, 1).reshape(F, -1).astype(bf)

    # slab tables + remapped int16 indices (flat n-major edge order)
    idxcols = np.zeros((16, e_pad // 16), np.int16)
    in_map = {}
    for s, (t0, t1) in enumerate([(0, nt_a), (nt_a, nt)]):
        sl = src[t0 * 128:t1 * 128]
        uniq, inv = np.unique(sl, return_inverse=True)
        assert len(uniq) <= 32767, f"slab {s} unique {len(uniq)}"
        in_map[f"tab{s}"] = np.ascontiguousarray(atom_fea[uniq]).astype(bf)
        jj = t0 * 1536 + np.arange(sl.size)
        idxcols[jj % 16, jj // 16] = inv.reshape(-1).astype(np.int16)
    idx16 = np.tile(idxcols, (8, 1))

    atomT = np.zeros((P, npad), np.float32)
    atomT[:, :nloc] = atom_fea[n0:n0 + nloc].T

    w1blk = np.zeros((P, P), np.float32)
    w1blk[:F, :F] = w1
    w1blk[F:, F:] = w1
    w2c = (w2[:, 0] * ALPHA).astype(np.float32)
    w2rep = np.tile(np.concatenate([w2c, w2c]).reshape(P, 1), (1, P))
    cbf = np.concatenate([w1blk, w2rep], axis=1).astype(bf)
    cf32 = np.concatenate(
        [np.stack([np.concatenate([b1, b1]), bn_gamma, bn_beta], axis=1),
         tp_w.astype(np.float32)], axis=1).astype(np.float32)

    in_map.update(nbrT=np.ascontiguousarray(nbrT), idx16=idx16, atomT=atomT,
                  cbf=np.ascontiguousarray(cbf), cf32=cf32)
    return in_map


def kernel(atom_fea, nbr_fea, nbr_idx, pos, w1, b1, w2, b2, tp_w,
           bn_gamma, bn_beta, _reps=1, _nc_cache={}):
    from concourse.bass_utils import run_bass_kernel_spmd

    atom_fea = np.asarray(atom_fea, np.float32)
    nbr_fea = np.asarray(nbr_fea, np.float32)
    nbr_idx = np.asarray(nbr_idx)
    w1 = np.asarray(w1, np.float32); b1 = np.asarray(b1, np.float32)
    w2 = np.asarray(w2, np.float32); b2 = np.asarray(b2, np.float32)
    tp_w = np.asarray(tp_w, np.float32)
    bn_gamma = np.asarray(bn_gamma, np.float32)
    bn_beta = np.asarray(bn_beta, np.float32)

    nloc = atom_fea.shape[0] // N_CORES
    in_maps = [prep_core_inputs(c, nloc, atom_fea, nbr_fea, nbr_idx, w1, b1,
                                w2, b2, tp_w, bn_gamma, bn_beta)
               for c in range(N_CORES)]
    n_slabs = sum(1 for k in in_maps[0] if k.startswith("tab"))
    u_max = [max(im[f"tab{s}"].shape[0] for im in in_maps) for s in range(n_slabs)]
    for im in in_maps:
        for s in range(n_slabs):
            t = im[f"tab{s}"]
            if t.shape[0] < u_max[s]:
                im[f"tab{s}"] = np.concatenate(
                    [t, np.zeros((u_max[s] - t.shape[0], C), t.dtype)])

    b2c = float(b2[0]) * float(ALPHA)
    key = (nloc, tuple(u_max), _reps)
    if key not in _nc_cache:
        _nc_cache[key] = build_bass(nloc, u_max, b2c, reps=_reps)
    nc = _nc_cache[key]
    res = run_bass_kernel_spmd(nc, in_maps, list(range(N_CORES)))
    out = np.concatenate(
        [res.results[c]["outT"][:, :nloc].T for c in range(N_CORES)], axis=0)
    return np.ascontiguousarray(out)



# revision 20
# speedup vs baseline: 19.5004x; 19.5004x over previous
"""Trainium2 Bass kernel for nn_E3ConvLayer (gnn_message_passing).

The reference reduces to (l>=1 spherical harmonics are dead code — only
W[:, :1] is used and Y[:, 0] == 1/sqrt(4*pi) is a constant; pos is unused):

  r0(e)  = softplus(nbr_fea[e] @ w1 + b1) @ w2[:, 0] + b2[0]
  w(e)   = r0(e) / (sqrt(4*pi) * sqrt(C) * M)
  pre[n] = sum_m w(n, m) * atom_fea[nbr_idx[n, m]]
  x      = atom_fea + pre @ tp_w
  out    = softplus(gamma * (x - mean(x)) * rsqrt(var(x) + 1e-5) + beta)

Design v2 (8 cores, nodes sharded 6250/core, padded to 6272 = 49*128):
  * edges permuted per 128-node tile to m-major order
    (e' = t*1536 + m*128 + n) so the per-node mean-reduce becomes 12
    accumulating PE matmuls with tp_w as lhsT (no DVE TensorReduce).
  * radial MLP on PE: 2x64 block-diagonal packing -> K=128 matmuls, N=512;
    softplus as Ln(Exp(x + b1) + 1), e-stream bf16 so Ln runs 2x.
  * all activations forced into one ACT table set (natural_log_exp) to
    kill LoadActFuncSet thrash.
  * neighbor features arrive TRANSPOSED via dma_gather(transpose=True)
    from per-slab index-remapped bf16 tables (int16 index limit).
  * weighting on DVE: gt = (r0 + b2c) * gt in place per 1024-edge chunk.
  * x = atomT + agg fused per 512-node group with accumulated BN stats
    (sum via DVE stt accum, sum-of-squares via ACT Square accum);
    stats AllReduce'd (2x128 floats); final softplus(gamma' x + beta')
    as one Exp + one Ln over the whole shard.
Host pre-transposes/pads/permutes inputs and post-transposes the output.
"""

import os
import sys
import numpy as np

sys.path.insert(0, "/opt/trn_rl_repo")

N_TOTAL, M, C, F = 50000, 12, 128, 64
N_CORES = 8
P = 128

_SQRT4PI = float(np.sqrt(4.0 * np.pi))
ALPHA = np.float32(1.0 / (_SQRT4PI * np.sqrt(C) * M))

_DBG = set(os.environ.get("KDBG", "").split(","))


def _plan(nloc):
    """Static shapes/chunking for one core."""
    npad = ((nloc + 127) // 128) * 128
    nt = npad // 128                  # 128-node tiles
    e_pad = npad * M                  # real edge slots (tile-m-major)
    ngrp = (e_pad + 4095) // 4096     # 4096-edge radial groups (2048 cols)
    nt_a = 4 * (nt // 8) if nt >= 8 else max(1, nt // 2)
    chunks = []                       # (slab, tile_start, ntiles)
    for s, (t0, t1) in enumerate([(0, nt_a), (nt_a, nt)]):
        ts = t0
        first = (s == 0)
        while ts < t1:
            n = min(2 if first else 4, t1 - ts)   # small first chunk: fast start
            first = False
            chunks.append((s, ts, n))
            ts += n
    return dict(npad=npad, nt=nt, e_pad=e_pad, ngrp=ngrp, nt_a=nt_a,
                chunks=chunks)


def _patch_act_tables(arch):
    """Force Exp/Ln to resolve to the one table set containing both, so the
    act-table-load pass emits a single load instead of thrashing."""
    import concourse.hw_specs as hw_specs
    from concourse import mybir
    tabs = hw_specs.get_activation_tables(arch)   # functools.cache'd dict
    exp = mybir.ActivationFunctionType.Exp
    ln = mybir.ActivationFunctionType.Ln
    both = [n for n, s in tabs.items() if exp in s and ln in s]
    if not both:
        return
    keep = both[0]
    for name, s in tabs.items():
        if name != keep:
            s.discard(exp)
            s.discard(ln)


def build_bass(nloc, u_sizes, b2c, reps=1, num_devices=N_CORES,
               hw_loop=False):
    import concourse.bacc as bacc
    import concourse.tile as tile
    from concourse import mybir
    from contextlib import ExitStack

    f32 = mybir.dt.float32
    bf16 = mybir.dt.bfloat16
    i16 = mybir.dt.int16
    AF = mybir.ActivationFunctionType
    ALU = mybir.AluOpType

    pl = _plan(nloc)
    npad, e_pad, ngrp = pl["npad"], pl["e_pad"], pl["ngrp"]
    chunks = pl["chunks"]

    nc = bacc.Bacc("TRN2", target_bir_lowering=False, debug=False,
                   enable_asserts=True, num_devices=num_devices)
    _patch_act_tables(nc.m.arch)

    # ---- DRAM parameters -------------------------------------------------
    nbrT = nc.dram_tensor("nbrT", [P, ngrp * 2048], bf16, kind="ExternalInput").ap()
    tabs = [nc.dram_tensor(f"tab{s}", [u_sizes[s], C], bf16, kind="ExternalInput").ap()
            for s in range(len(u_sizes))]
    idx16 = nc.dram_tensor("idx16", [P, e_pad // 16], i16, kind="ExternalInput").ap()
    atomT = nc.dram_tensor("atomT", [P, npad], bf16, kind="ExternalInput").ap()
    cbf = nc.dram_tensor("cbf", [P, 384], bf16, kind="ExternalInput").ap()
    cf32 = nc.dram_tensor("cf32", [P, 3], f32, kind="ExternalInput").ap()
    outT = nc.dram_tensor("outT", [P, npad], bf16, kind="ExternalOutput").ap()
    cc_in = nc.dram_tensor("cc_in", [P, 2], f32).ap()
    cc_out = nc.dram_tensor("cc_out", [P, 2], f32, addr_space="Shared").ap()

    with tile.TileContext(nc) as tc, ExitStack() as ctx:
        const_pool = ctx.enter_context(tc.tile_pool(name="const", bufs=1))
        nbr_pool = ctx.enter_context(tc.tile_pool(name="nbr", bufs=2))
        e_pool = ctx.enter_context(tc.tile_pool(name="epak", bufs=2))
        s_pool = ctx.enter_context(tc.tile_pool(name="spak", bufs=4))
        g_pool = ctx.enter_context(tc.tile_pool(name="gath", bufs=2))
        big_pool = ctx.enter_context(tc.tile_pool(name="big", bufs=1))
        stat_pool = ctx.enter_context(tc.tile_pool(name="stat", bufs=1))

        # h (radial pre-act) and r0 share one rotating pool of [P,1024]
        # banks: 3x2 + agg 2x1 = 8 PSUM banks
        psum_hr = ctx.enter_context(tc.tile_pool(name="phr", bufs=3, space="PSUM"))
        psum_agg = ctx.enter_context(tc.tile_pool(name="pagg", bufs=2, space="PSUM"))

        # ---- constant preloads (first chunk's indices first) ------------
        idx_sb = const_pool.tile([P, e_pad // 16], i16)
        i0 = (chunks[0][2] * 1536) // 16     # first chunk's index slice first
        nc.sync.dma_start(idx_sb[:, :i0], idx16[:, :i0])
        cb_sb = const_pool.tile([P, 384], bf16)
        nc.sync.dma_start(cb_sb[:], cbf[:])
        w1_sb = cb_sb[:, 0:128]          # blockdiag(w1, w1)
        w2_sb = cb_sb[:, 128:256]        # [w2c*alpha] replicated to 128 cols, x2
        tpw_sb = cb_sb[:, 256:384]       # tp_w (bf16)
        cf_sb = const_pool.tile([P, 3], f32)
        nc.sync.dma_start(cf_sb[:], cf32[:])
        b1_sb = cf_sb[:, 0:1]
        gam_sb = cf_sb[:, 1:2]
        bet_sb = cf_sb[:, 2:3]

        x_sb = big_pool.tile([P, npad], f32, tag="x")

        def emit_rep(first, use_cc):
            stats = stat_pool.tile([P, 32], f32, tag="st")
            sq_scr = stat_pool.tile([P, 512], f32, tag="sq")
            atom_sb = big_pool.tile([P, npad], bf16, tag="atom")

            # radial MLP groups are emitted lazily (ring of s-tiles) just
            # before the edge-chunks that consume them
            s_tiles = {}
            next_g = [0]

            def emit_radial_through(g_need):
                while next_g[0] <= g_need:
                    g = next_g[0]
                    nbr_sb = nbr_pool.tile([P, 2048], bf16, tag="nbr")
                    nc.sync.dma_start(nbr_sb[:], nbrT[:, g * 2048:(g + 1) * 2048])
                    e_sb = e_pool.tile([P, 2048], bf16, tag="e")
                    for b in range(2):
                        h_ps = psum_hr.tile([P, 1024], f32, tag="hr")
                        for q in range(2):
                            nc.tensor.matmul(
                                h_ps[:, q * 512:(q + 1) * 512], w1_sb,
                                nbr_sb[:, b * 1024 + q * 512:b * 1024 + (q + 1) * 512],
                                start=True, stop=True)
                        nc.scalar.activation(e_sb[:, b * 1024:(b + 1) * 1024],
                                             h_ps[:], AF.Exp, bias=b1_sb)
                    s_sb = s_pool.tile([P, 2048], bf16, tag="s")
                    nc.scalar.activation(s_sb[:], e_sb[:], AF.Ln, bias=1.0)
                    s_tiles[g] = s_sb
                    next_g[0] += 1

            # radial group 0 ahead of the bulk index load so the first
            # chunk's r0 chain starts immediately
            emit_radial_through(0)
            if first:
                nc.sync.dma_start(idx_sb[:, i0:], idx16[:, i0:])

            # ====== gather + r0-weight (DVE) + agg via PE fold ===========
            for ci, (slab, ts, gnt) in enumerate(chunks):
                ne = gnt * 1536
                e0 = ts * 1536
                gath = g_pool.tile([P, 6144], bf16, tag="g")
                gt = gath[:, :ne]
                if "nogather" in _DBG:
                    nc.vector.memset(gt, 1.0)
                else:
                    gv = gt.rearrange("p (k d) -> p k d", d=ne)
                    nc.gpsimd.dma_gather(
                        out_ap=gv, in_ap=tabs[slab][:],
                        idxs_ap=idx_sb[:, e0 // 16:(e0 + ne) // 16],
                        num_idxs=ne, num_idxs_reg=ne,
                        elem_size=C, transpose=True, single_packet=False)
                emit_radial_through(min((e0 + ne - 1) // 4096, ngrp - 1))
                # r0 per 1024-edge block, multiply into gathered stream
                for c in range((ne + 1023) // 1024):
                    cw = min(1024, ne - c * 1024)
                    B = (e0 + c * 1024) // 1024        # global 1024-edge block
                    r0_ps = psum_hr.tile([P, 1024], f32, tag="hr")
                    s_sb = s_tiles[B // 4]
                    scol = (B % 4) * 512
                    for half in range(cw // 512):
                        nc.tensor.matmul(
                            r0_ps[:, half * 512:(half + 1) * 512],
                            w2_sb[64 * half:64 * (half + 1), :],
                            s_sb[64 * half:64 * (half + 1), scol:scol + 512],
                            start=True, stop=True)
                    off = c * 1024
                    nc.vector.scalar_tensor_tensor(
                        out=gt[:, off:off + cw], in0=r0_ps[:, :cw],
                        scalar=float(b2c), in1=gt[:, off:off + cw],
                        op0=ALU.add, op1=ALU.mult)
                if ci == 0:
                    # deferred past the startup-critical nbr/idx DMAs but
                    # emitted before the first x-stt reads it
                    nc.sync.dma_start(atom_sb[:], atomT[:])
                # agg^T tile group = sum_m tp_w^T @ Gw[:, m-block]
                w = gnt * 128
                n0 = ts * 128
                agg_ps = psum_agg.tile([P, 512], f32, tag="agg")
                gvm = gt.rearrange("p (t m n) -> p m t n", m=M, n=128)
                for m in range(M):
                    nc.tensor.matmul(agg_ps[:, :w], tpw_sb, gvm[:, m:m + 1],
                                     start=(m == 0), stop=(m == M - 1))
                # x = atom + agg ; accumulate BN stats per group
                valid = max(0, min(nloc - n0, w))
                if valid:
                    nc.vector.scalar_tensor_tensor(
                        out=x_sb[:, n0:n0 + valid], in0=agg_ps[:, :valid],
                        scalar=1.0, in1=atom_sb[:, n0:n0 + valid],
                        op0=ALU.mult, op1=ALU.add,
                        accum_out=stats[:, ci:ci + 1])
                    nc.scalar.activation(sq_scr[:, :valid], x_sb[:, n0:n0 + valid],
                                         AF.Square,
                                         accum_out=stats[:, 16 + ci:17 + ci])
                if valid < w:
                    nc.vector.scalar_tensor_tensor(
                        out=x_sb[:, n0 + valid:n0 + w],
                        in0=agg_ps[:, valid:w], scalar=1.0,
                        in1=atom_sb[:, n0 + valid:n0 + w],
                        op0=ALU.mult, op1=ALU.add)

            ncc = len(chunks)
            cc_sb = stat_pool.tile([P, 2], f32, tag="cc")
            nc.vector.reduce_sum(cc_sb[:, 0:1],
                                 stats[:, 0:ncc].rearrange("p (a c) -> p a c", a=1),
                                 axis=mybir.AxisListType.X)
            nc.vector.reduce_sum(cc_sb[:, 1:2],
                                 stats[:, 16:16 + ncc].rearrange("p (a c) -> p a c", a=1),
                                 axis=mybir.AxisListType.X)

            # ---- global BN stats over all cores ----
            nc.sync.dma_start(cc_in[:], cc_sb[:])
            if not use_cc:
                nc.sync.dma_start(cc_out[:], cc_in[:])
            else:
                nc.gpsimd.collective_compute(
                    "AllReduce", ALU.add,
                    replica_groups=[list(range(num_devices))],
                    ins=[cc_in[:]], outs=[cc_out[:]])
            st_sb = stat_pool.tile([P, 2], f32, tag="stg")
            nc.sync.dma_start(st_sb[:], cc_out[:])

            # mean = sx/N ; var = sq/N - mean^2 ; rstd = exp(-0.5 ln(var+eps))
            w_sb = stat_pool.tile([P, 8], f32, tag="w")
            inv_n = 1.0 / float(nloc * num_devices)
            nc.vector.tensor_scalar_mul(w_sb[:, 0:1], st_sb[:, 0:1], inv_n)
            nc.vector.tensor_scalar_mul(w_sb[:, 1:2], st_sb[:, 1:2], inv_n)
            nc.vector.scalar_tensor_tensor(
                out=w_sb[:, 2:3], in0=w_sb[:, 0:1], scalar=-1.0,
                in1=w_sb[:, 0:1], op0=ALU.mult, op1=ALU.mult)  # -mean^2
            nc.vector.scalar_tensor_tensor(
                out=w_sb[:, 1:2], in0=w_sb[:, 1:2], scalar=1e-5,
                in1=w_sb[:, 2:3], op0=ALU.add, op1=ALU.add)    # var + eps
            nc.scalar.activation(w_sb[:, 2:3], w_sb[:, 1:2], AF.Ln)
            nc.scalar.activation(w_sb[:, 3:4], w_sb[:, 2:3], AF.Exp, scale=-0.5)
            # Newton: y <- y * (1.5 - 0.5*(var+eps)*y^2)
            nc.vector.tensor_tensor(w_sb[:, 4:5], w_sb[:, 3:4], w_sb[:, 3:4], op=ALU.mult)
            nc.vector.tensor_tensor(w_sb[:, 4:5], w_sb[:, 4:5], w_sb[:, 1:2], op=ALU.mult)
            nc.vector.tensor_scalar(w_sb[:, 4:5], w_sb[:, 4:5], -0.5, 1.5,
                                    op0=ALU.mult, op1=ALU.add)
            nc.vector.tensor_tensor(w_sb[:, 3:4], w_sb[:, 3:4], w_sb[:, 4:5], op=ALU.mult)
            # gamma' = gamma * rstd ; beta' = beta - mean * gamma'
            nc.vector.tensor_tensor(w_sb[:, 5:6], gam_sb, w_sb[:, 3:4], op=ALU.mult)
            nc.vector.tensor_tensor(w_sb[:, 6:7], w_sb[:, 0:1], w_sb[:, 5:6], op=ALU.mult)
            nc.vector.tensor_tensor(w_sb[:, 6:7], bet_sb, w_sb[:, 6:7], op=ALU.subtract)

            # ---- final: out = softplus(gamma' * x + beta') ----
            # sliced so Exp/Ln/DMA pipeline instead of serializing
            e2_sb = big_pool.tile([P, npad], f32, tag="e2")
            o_sb = big_pool.tile([P, npad], bf16, tag="o")
            nsl = 4
            sw = npad // nsl
            for si in range(nsl):
                sl = slice(si * sw, (si + 1) * sw if si < nsl - 1 else npad)
                nc.scalar.activation(e2_sb[:, sl], x_sb[:, sl], AF.Exp,
                                     bias=w_sb[:, 6:7], scale=w_sb[:, 5:6])
                nc.scalar.activation(o_sb[:, sl], e2_sb[:, sl], AF.Ln, bias=1.0)
                nc.sync.dma_start(outT[:, sl], o_sb[:, sl])

        use_cc = "nocc" not in _DBG
        if hw_loop:
            # timing mode: same per-rep body inside a hardware loop; the
            # collective is replaced by its DRAM round-trip stand-in
            # (collectives are not allowed inside control flow)
            nc.sync.dma_start(idx_sb[:, i0:], idx16[:, i0:])
            with tc.For_i(0, reps):
                emit_rep(first=False, use_cc=False)
        else:
            for rep in range(reps):
                emit_rep(first=(rep == 0), use_cc=use_cc)

    nc.compile()
    return nc


def prep_core_inputs(core, nloc, atom_fea, nbr_fea, nbr_idx, w1, b1, w2, b2,
                     tp_w, bn_gamma, bn_beta):
    """Host-side layout prep for one core."""
    import ml_dtypes
    bf = ml_dtypes.bfloat16

    pl = _plan(nloc)
    npad, e_pad, ngrp = pl["npad"], pl["e_pad"], pl["ngrp"]
    nt, nt_a = pl["nt"], pl["nt_a"]

    n0 = core * nloc
    nbr = np.zeros((npad, M, F), np.float32)
    nbr[:nloc] = nbr_fea[n0:n0 + nloc]
    src = np.zeros((npad, M), np.int64)
    src[:nloc] = nbr_idx[n0:n0 + nloc]

    # permute edges to per-tile m-major order: e' = t*1536 + m*128 + n
    perm = np.arange(e_pad).reshape(nt, 128, M).transpose(0, 2, 1).reshape(-1)

    # radial stream (permuted edges), packed 2x64 rows per 1024-edge block
    X = nbr.reshape(e_pad, F)[perm]
    Xp = np.zeros((ngrp * 4096, F), np.float32)
    Xp[:e_pad] = X
    blocks = Xp.reshape(ngrp * 4, 2, 512, F)
    nbrT = np.empty((P, ngrp * 2048), bf)
    nbrT[:F] = blocks[:, 0].transpose(2, 0, 1).reshape(F, -1).astype(bf)
    nbrT[F:] = blocks[:, 1].transpose(2, 0, 1).reshape(F, -1).astype(bf)

    # slab tables + remapped int16 indices (permuted edge order)
    src_p = src.reshape(-1)[perm]
    idxcols = np.zeros((16, e_pad // 16), np.int16)
    in_map = {}
    for s, (t0, t1) in enumerate([(0, nt_a), (nt_a, nt)]):
        sl = src_p[t0 * 1536:t1 * 1536]
        uniq, inv = np.unique(sl, return_inverse=True)
        assert len(uniq) <= 32767, f"slab {s} unique {len(uniq)}"
        in_map[f"tab{s}"] = np.ascontiguousarray(atom_fea[uniq]).astype(bf)
        jj = t0 * 1536 + np.arange(sl.size)
        idxcols[jj % 16, jj // 16] = inv.reshape(-1).astype(np.int16)
    idx16 = np.tile(idxcols, (8, 1))

    atomT = np.zeros((P, npad), bf)
    atomT[:, :nloc] = atom_fea[n0:n0 + nloc].T.astype(bf)

    w1blk = np.zeros((P, P), np.float32)
    w1blk[:F, :F] = w1
    w1blk[F:, F:] = w1
    w2c = (w2[:, 0] * ALPHA).astype(np.float32)
    w2rep = np.tile(np.concatenate([w2c, w2c]).reshape(P, 1), (1, P))
    cbf = np.concatenate([w1blk, w2rep, tp_w.astype(np.float32)],
                         axis=1).astype(bf)
    cf32 = np.stack([np.concatenate([b1, b1]), bn_gamma, bn_beta],
                    axis=1).astype(np.float32)

    in_map.update(nbrT=np.ascontiguousarray(nbrT), idx16=idx16, atomT=atomT,
                  cbf=np.ascontiguousarray(cbf), cf32=cf32)
    return in_map


def kernel(atom_fea, nbr_fea, nbr_idx, pos, w1, b1, w2, b2, tp_w,
           bn_gamma, bn_beta, _reps=1, _nc_cache={}):
    from concourse.bass_utils import run_bass_kernel_spmd

    atom_fea = np.asarray(atom_fea, np.float32)
    nbr_fea = np.asarray(nbr_fea, np.float32)
    nbr_idx = np.asarray(nbr_idx)
    w1 = np.asarray(w1, np.float32); b1 = np.asarray(b1, np.float32)
    w2 = np.asarray(w2, np.float32); b2 = np.asarray(b2, np.float32)
    tp_w = np.asarray(tp_w, np.float32)
    bn_gamma = np.asarray(bn_gamma, np.float32)
    bn_beta = np.asarray(bn_beta, np.float32)

    nloc = atom_fea.shape[0] // N_CORES
    in_maps = [prep_core_inputs(c, nloc, atom_fea, nbr_fea, nbr_idx, w1, b1,
                                w2, b2, tp_w, bn_gamma, bn_beta)
               for c in range(N_CORES)]
    n_slabs = sum(1 for k in in_maps[0] if k.startswith("tab"))
    u_max = [max(im[f"tab{s}"].shape[0] for im in in_maps) for s in range(n_slabs)]
    for im in in_maps:
        for s in range(n_slabs):
            t = im[f"tab{s}"]
            if t.shape[0] < u_max[s]:
                im[f"tab{s}"] = np.concatenate(
                    [t, np.zeros((u_max[s] - t.shape[0], C), t.dtype)])

    b2c = float(b2[0]) * float(ALPHA)
    key = (nloc, tuple(u_max), _reps)
    if key not in _nc_cache:
        _nc_cache[key] = build_bass(nloc, u_max, b2c, reps=_reps)
    nc = _nc_cache[key]
    res = run_bass_kernel_spmd(nc, in_maps, list(range(N_CORES)))
    out = np.concatenate(
        [res.results[c]["outT"][:, :nloc].T.astype(np.float32) for c in range(N_CORES)], axis=0)
    return np.ascontiguousarray(out)
